# revision 2
# baseline (speedup 1.0000x reference)
"""Grouped linear (MoE routed GEMM) on 8 Trainium2 NeuronCores.

out[t] = hidden_states[t] @ weight[g(t)] where g(t) is the expert owning
token t (contiguous groups sized by tokens_per_expert).

Strategy (expert-parallel, token-balanced):
  - All group sizes are multiples of 128, so work = 64 row-tiles of 128
    tokens. Each core gets exactly 8 row-tiles (1024 tokens).
  - SPMD requires one program for all cores, so every core runs the same
    static slot pattern [0,0,0,1,1,1,2,2]: 3 weight slots covering 3/3/2
    row-tiles. The host decomposes the per-expert tile counts into
    sixteen 3-tile parts + eight 2-tile parts and assigns (expert ->
    core,slot); each core receives its 1024 tokens pre-transposed plus a
    [3,1024,1024] stacked weight tensor.
  - Device: bf16 matmuls (fp32 PSUM accumulate), K-contiguous chains of
    8 matmuls per [128,512] output tile; DMA overlapped with compute.
"""

import numpy as np
import ml_dtypes
from contextlib import ExitStack

import concourse.bass as bass
import concourse.tile as tile
from concourse import bacc, mybir
from concourse.bass_utils import run_bass_kernel_spmd

T, D, G, NCORES = 8192, 1024, 8, 8
TPC = T // NCORES            # tokens per core
RT = TPC // 128              # row tiles per core (8)
KT = D // 128                # contraction tiles (8)
NSLOTS = 3
PATTERN = (0, 0, 0, 1, 1, 1, 2, 2)   # row-tile -> weight slot
SLOT_SIZES = (3, 3, 2)               # tiles per slot

CDT = mybir.dt.bfloat16      # compute dtype on device
NP_CDT = ml_dtypes.bfloat16
ODT = mybir.dt.float32       # output dtype from device
NP_ODT = np.float32

_PROG = None
LAST_RESULTS = None          # test harness reads exec_time_ns from here


def _build_program():
    nc = bacc.Bacc("TRN2", target_bir_lowering=False, debug=False,
                   num_devices=NCORES)
    xt_d = nc.dram_tensor("xt", [D, TPC], CDT, kind="ExternalInput")
    w_d = nc.dram_tensor("w", [NSLOTS, D, D], CDT, kind="ExternalInput")
    o_d = nc.dram_tensor("o", [TPC, D], ODT, kind="ExternalOutput")

    with tile.TileContext(nc) as tc, ExitStack() as ctx:
        xt_pool = ctx.enter_context(tc.tile_pool(name="xt", bufs=1))
        w_pool = ctx.enter_context(tc.tile_pool(name="w", bufs=1))
        ps_pool = ctx.enter_context(
            tc.tile_pool(name="ps", bufs=4, space=bass.MemorySpace.PSUM))
        out_pool = ctx.enter_context(tc.tile_pool(name="out", bufs=4))

        # Activations: 8 k-tiles, each [128, 1024] (fine-grained so matmul
        # chains can start before the whole tensor lands).
        xt_sb = []
        for k in range(KT):
            t = xt_pool.tile([128, TPC], CDT, tag=f"xt{k}")
            nc.sync.dma_start(t[:], xt_d[k * 128:(k + 1) * 128, :])
            xt_sb.append(t)

        w_sb = {}
        for s in range(NSLOTS):
            tiles = []
            for k in range(KT):
                t = w_pool.tile([128, D], CDT, tag=f"w{s}_{k}")
                nc.sync.dma_start(t[:], w_d[s, k * 128:(k + 1) * 128, :])
                tiles.append(t)
            w_sb[s] = tiles

            for rt in range(RT):
                if PATTERN[rt] != s:
                    continue
                for oh in range(2):  # two 512-wide output column halves
                    ps = ps_pool.tile([128, 512], mybir.dt.float32)
                    for k in range(KT):
                        nc.tensor.matmul(
                            ps[:],
                            xt_sb[k][:, rt * 128:(rt + 1) * 128],
                            w_sb[s][k][:, oh * 512:(oh + 1) * 512],
                            start=(k == 0),
                            stop=(k == KT - 1),
                        )
                    ot = out_pool.tile([128, 512], ODT)
                    nc.vector.tensor_copy(ot[:], ps[:])
                    nc.sync.dma_start(
                        o_d[rt * 128:(rt + 1) * 128, oh * 512:(oh + 1) * 512],
                        ot[:])

    nc.compile()
    return nc


def _get_program():
    global _PROG
    if _PROG is None:
        _PROG = _build_program()
    return _PROG


def _solve_parts(tiles_per_expert):
    """Decompose per-expert tile counts into 16 parts of 3 tiles and 8
    parts of 2 tiles. Returns (threes, twos) as lists of expert ids, or
    None if infeasible."""
    t = list(tiles_per_expert)
    f = [c % 2 for c in t]              # number of 3-parts per expert
    if any(3 * f[g] > t[g] for g in range(len(t))):
        return None
    h = [(t[g] - 3 * f[g]) // 2 for g in range(len(t))]
    # each f+=2 converts three 2-parts into two 3-parts
    while sum(h) > 8:
        g = max(range(len(t)), key=lambda i: h[i])
        if h[g] < 3:
            return None
        f[g] += 2
        h[g] -= 3
    if sum(h) != 8 or sum(f) != 16:
        return None
    threes, twos = [], []
    for g in range(len(t)):
        threes += [g] * f[g]
        twos += [g] * h[g]
    return threes, twos


def _numpy_fallback(hidden_states, weight, counts):
    out = np.empty((hidden_states.shape[0], weight.shape[2]), np.float32)
    start = 0
    for g in range(weight.shape[0]):
        end = start + int(counts[g])
        out[start:end] = hidden_states[start:end].astype(np.float32) @ \
            weight[g].astype(np.float32)
        start = end
    return out


def kernel(hidden_states, weight, tokens_per_expert):
    counts = np.asarray(tokens_per_expert).astype(np.int64)
    out_dtype = hidden_states.dtype

    ok = (hidden_states.shape == (T, D) and weight.shape == (G, D, D)
          and counts.shape == (G,) and counts.sum() == T
          and np.all(counts % 128 == 0) and np.all(counts >= 0))
    parts = _solve_parts(counts // 128) if ok else None
    if parts is None:
        return _numpy_fallback(hidden_states, weight, counts).astype(out_dtype)
    threes, twos = parts

    # Global preprocessing: transpose+cast activations once, cast weights.
    ht = np.ascontiguousarray(
        hidden_states.astype(NP_CDT).T)          # [D, T] bf16
    wc = weight.astype(NP_CDT)                    # [G, D, D] bf16

    # Per-expert global row offsets; consume tiles in order.
    expert_row = dict(
        (g, int(o)) for g, o in enumerate(np.concatenate(
            [[0], np.cumsum(counts)[:-1]])))

    in_maps = []
    core_rows = []       # per core: list of (global_row_start, n_rows)
    for c in range(NCORES):
        part_list = [(threes[2 * c], 3 * 128), (threes[2 * c + 1], 3 * 128),
                     (twos[c], 2 * 128)]
        spans = []
        for g, nrows in part_list:
            r0 = expert_row[g]
            expert_row[g] = r0 + nrows
            spans.append((r0, nrows))
        core_rows.append(spans)
        xt_c = np.concatenate(
            [ht[:, r0:r0 + n] for r0, n in spans], axis=1)
        xt_c = np.ascontiguousarray(xt_c)
        w_c = np.ascontiguousarray(
            np.stack([wc[g] for g, _ in part_list]))
        in_maps.append({"xt": xt_c, "w": w_c})

    nc = _get_program()
    global LAST_RESULTS
    LAST_RESULTS = run_bass_kernel_spmd(nc, in_maps, list(range(NCORES)))

    out = np.empty((T, D), np.float32)
    for c in range(NCORES):
        o_c = np.asarray(LAST_RESULTS.results[c]["o"], dtype=np.float32)
        r = 0
        for r0, n in core_rows[c]:
            out[r0:r0 + n] = o_c[r:r + n]
            r += n
    return out.astype(out_dtype, copy=False)


# revision 4
# speedup vs baseline: 1.0925x; 1.0925x over previous
"""Grouped linear (MoE routed GEMM) on 8 Trainium2 NeuronCores.

out[t] = hidden_states[t] @ weight[g(t)] where g(t) is the expert owning
token t (contiguous groups sized by tokens_per_expert).

Strategy (expert-parallel, token-balanced):
  - All group sizes are multiples of 128, so work = 64 row-tiles of 128
    tokens. Each core gets exactly 8 row-tiles (1024 tokens).
  - SPMD requires one program for all cores, so every core runs the same
    static slot pattern [0,0,0,1,1,1,2,2]: 3 weight slots covering 3/3/2
    row-tiles. The host decomposes the per-expert tile counts into
    sixteen 3-tile parts + eight 2-tile parts and assigns (expert ->
    core,slot); each core receives its 1024 tokens pre-transposed plus a
    [3,1024,1024] stacked weight tensor.
  - Device: bf16 matmuls (fp32 PSUM accumulate), K-contiguous chains of
    8 matmuls per [128,512] output tile. Loads are emitted in exact
    consume order and rate-limited by a dependency ladder so tiles
    arrive progressively (SDMA round-robins across queues at packet
    granularity; without the ladder every transfer lands at the same
    late time and the PE idles for ~12us). Stores go on the scalar
    engine's HWDGE ring so they never stall behind load waits.
"""

import os
import numpy as np
import ml_dtypes
from contextlib import ExitStack

import concourse.bass as bass
import concourse.tile as tile
from concourse import bacc, mybir
from concourse.bass_utils import run_bass_kernel_spmd
from concourse.tile import add_dep_helper

T, D, G, NCORES = 8192, 1024, 8, 8
TPC = T // NCORES            # tokens per core
RT = TPC // 128              # row tiles per core (8)
KT = D // 128                # contraction tiles (8)
NSLOTS = 3
PATTERN = (0, 0, 0, 1, 1, 1, 2, 2)   # row-tile -> weight slot
LADDER_DEPTH = int(os.environ.get("K_LADDER", "4"))
WARMUP_MMS = int(os.environ.get("K_WARMUP", "6"))

CDT = mybir.dt.bfloat16      # compute dtype on device
NP_CDT = ml_dtypes.bfloat16
ODT = mybir.dt.bfloat16      # device output dtype (host upcasts)
NP_ODT = ml_dtypes.bfloat16

_PROG = None
LAST_RESULTS = None          # test harness reads exec_time_ns from here


def _ins(x):
    return getattr(x, "ins", x)


def _build_program():
    nc = bacc.Bacc("TRN2", target_bir_lowering=False, debug=False,
                   num_devices=NCORES)
    xt_d = nc.dram_tensor("xt", [D, TPC], CDT, kind="ExternalInput")
    w_d = nc.dram_tensor("w", [NSLOTS, D, D], CDT, kind="ExternalInput")
    o_d = nc.dram_tensor("o", [TPC, D], ODT, kind="ExternalOutput")

    with tile.TileContext(nc) as tc, ExitStack() as ctx:
        xt_pool = ctx.enter_context(tc.tile_pool(name="xt", bufs=1))
        w_pool = ctx.enter_context(tc.tile_pool(name="w", bufs=1))
        ps_pool = ctx.enter_context(
            tc.tile_pool(name="ps", bufs=4, space=bass.MemorySpace.PSUM))
        out_pool = ctx.enter_context(tc.tile_pool(name="out", bufs=4))
        warm_pool = ctx.enter_context(tc.tile_pool(name="warm", bufs=1))

        # --- PE warmup: junk matmuls so HAM un-throttles while the first
        # real tiles are still in flight.
        if WARMUP_MMS:
            wt = warm_pool.tile([128, 512], CDT, tag="warm_sb")
            nc.gpsimd.memset(wt[:], 0)
            wps_pool = ctx.enter_context(
                tc.tile_pool(name="wps", bufs=1, space=bass.MemorySpace.PSUM))
            wps = wps_pool.tile([128, 512], mybir.dt.float32, tag="warm_ps")
            for _ in range(WARMUP_MMS):
                nc.tensor.matmul(wps[:], wt[:, 0:128], wt[:],
                                 start=True, stop=True)

        # --- Loads, in exact consume order, rate-limited by a ladder.
        load_insts = []

        def ladder_dma(dst_ap, src_ap):
            inst = nc.sync.dma_start(dst_ap, src_ap)
            if len(load_insts) >= LADDER_DEPTH:
                add_dep_helper(_ins(inst), _ins(load_insts[-LADDER_DEPTH]),
                               sync=True, reason="load ladder")
            load_insts.append(inst)
            return inst

        xt_sb = [None] * KT
        w_sb = {s: [None] * KT for s in range(NSLOTS)}

        def load_xt(k):
            t = xt_pool.tile([128, TPC], CDT, tag=f"xt{k}")
            ladder_dma(t[:], xt_d[k * 128:(k + 1) * 128, :])
            xt_sb[k] = t

        def load_w(s, k):
            t = w_pool.tile([128, D], CDT, tag=f"w{s}_{k}")
            ladder_dma(t[:], w_d[s, k * 128:(k + 1) * 128, :])
            w_sb[s][k] = t

        for k in range(KT):          # wave 0: xt and w0 interleaved
            load_xt(k)
            load_w(0, k)
        for s in range(1, NSLOTS):   # later slots follow in the ladder
            for k in range(KT):
                load_w(s, k)

        # --- Compute: per row tile, two 512-wide output halves, each an
        # 8-matmul K-chain into one PSUM bank.
        for rt in range(RT):
            s = PATTERN[rt]
            ot = out_pool.tile([128, D], ODT, tag="ot")
            for oh in range(2):
                ps = ps_pool.tile([128, 512], mybir.dt.float32, tag="ps")
                for k in range(KT):
                    nc.tensor.matmul(
                        ps[:],
                        xt_sb[k][:, rt * 128:(rt + 1) * 128],
                        w_sb[s][k][:, oh * 512:(oh + 1) * 512],
                        start=(k == 0),
                        stop=(k == KT - 1),
                    )
                nc.vector.tensor_copy(ot[:, oh * 512:(oh + 1) * 512], ps[:])
            # store on the scalar HWDGE ring so it can't stall load waits
            nc.scalar.dma_start(o_d[rt * 128:(rt + 1) * 128, :], ot[:])

    nc.compile()
    return nc


def _get_program():
    global _PROG
    if _PROG is None:
        _PROG = _build_program()
    return _PROG


def _solve_parts(tiles_per_expert):
    """Decompose per-expert tile counts into 16 parts of 3 tiles and 8
    parts of 2 tiles. Returns (threes, twos) as lists of expert ids, or
    None if infeasible."""
    t = list(tiles_per_expert)
    f = [c % 2 for c in t]              # number of 3-parts per expert
    if any(3 * f[g] > t[g] for g in range(len(t))):
        return None
    h = [(t[g] - 3 * f[g]) // 2 for g in range(len(t))]
    # each f+=2 converts three 2-parts into two 3-parts
    while sum(h) > 8:
        g = max(range(len(t)), key=lambda i: h[i])
        if h[g] < 3:
            return None
        f[g] += 2
        h[g] -= 3
    if sum(h) != 8 or sum(f) != 16:
        return None
    threes, twos = [], []
    for g in range(len(t)):
        threes += [g] * f[g]
        twos += [g] * h[g]
    return threes, twos


def _numpy_fallback(hidden_states, weight, counts):
    out = np.empty((hidden_states.shape[0], weight.shape[2]), np.float32)
    start = 0
    for g in range(weight.shape[0]):
        end = start + int(counts[g])
        out[start:end] = hidden_states[start:end].astype(np.float32) @ \
            weight[g].astype(np.float32)
        start = end
    return out


def kernel(hidden_states, weight, tokens_per_expert):
    counts = np.asarray(tokens_per_expert).astype(np.int64)
    out_dtype = hidden_states.dtype

    ok = (hidden_states.shape == (T, D) and weight.shape == (G, D, D)
          and counts.shape == (G,) and counts.sum() == T
          and np.all(counts % 128 == 0) and np.all(counts >= 0))
    parts = _solve_parts(counts // 128) if ok else None
    if parts is None:
        return _numpy_fallback(hidden_states, weight, counts).astype(out_dtype)
    threes, twos = parts

    # Global preprocessing: transpose+cast activations once, cast weights.
    ht = np.ascontiguousarray(
        hidden_states.astype(NP_CDT).T)          # [D, T] bf16
    wc = weight.astype(NP_CDT)                    # [G, D, D] bf16

    # Per-expert global row offsets; consume tiles in order.
    expert_row = dict(
        (g, int(o)) for g, o in enumerate(np.concatenate(
            [[0], np.cumsum(counts)[:-1]])))

    in_maps = []
    core_rows = []       # per core: list of (global_row_start, n_rows)
    for c in range(NCORES):
        part_list = [(threes[2 * c], 3 * 128), (threes[2 * c + 1], 3 * 128),
                     (twos[c], 2 * 128)]
        spans = []
        for g, nrows in part_list:
            r0 = expert_row[g]
            expert_row[g] = r0 + nrows
            spans.append((r0, nrows))
        core_rows.append(spans)
        xt_c = np.concatenate(
            [ht[:, r0:r0 + n] for r0, n in spans], axis=1)
        xt_c = np.ascontiguousarray(xt_c)
        w_c = np.ascontiguousarray(
            np.stack([wc[g] for g, _ in part_list]))
        in_maps.append({"xt": xt_c, "w": w_c})

    nc = _get_program()
    global LAST_RESULTS
    LAST_RESULTS = run_bass_kernel_spmd(nc, in_maps, list(range(NCORES)))

    out = np.empty((T, D), np.float32)
    for c in range(NCORES):
        o_c = np.asarray(LAST_RESULTS.results[c]["o"]).astype(np.float32)
        r = 0
        for r0, n in core_rows[c]:
            out[r0:r0 + n] = o_c[r:r + n]
            r += n
    return out.astype(out_dtype, copy=False)


# revision 5
# speedup vs baseline: 1.1236x; 1.0285x over previous
"""Grouped linear (MoE routed GEMM) on 8 Trainium2 NeuronCores.

out[t] = hidden_states[t] @ weight[g(t)] where g(t) is the expert owning
token t (contiguous groups sized by tokens_per_expert).

Strategy (expert-parallel, token-balanced):
  - All group sizes are multiples of 128, so work = 64 row-tiles of 128
    tokens. Each core gets exactly 8 row-tiles (1024 tokens).
  - SPMD requires one program for all cores, so every core runs the same
    static slot pattern [0,0,0,1,1,1,2,2]: 3 weight slots covering 3/3/2
    row-tiles. The host decomposes the per-expert tile counts into
    sixteen 3-tile parts + eight 2-tile parts and assigns (expert ->
    core,slot); each core receives its 1024 tokens pre-transposed plus a
    [3,1024,1024] stacked weight tensor.
  - Device: bf16 matmuls (fp32 PSUM accumulate), K-contiguous chains of
    8 matmuls per [128,512] output tile. Loads are emitted in exact
    consume order and rate-limited by a dependency ladder so tiles
    arrive progressively (SDMA round-robins across queues at packet
    granularity; without the ladder every transfer lands at the same
    late time and the PE idles for ~12us). Stores go on the scalar
    engine's HWDGE ring so they never stall behind load waits.
"""

import os
import numpy as np
import ml_dtypes
from contextlib import ExitStack

import concourse.bass as bass
import concourse.tile as tile
from concourse import bacc, mybir
from concourse.bass_utils import run_bass_kernel_spmd
from concourse.tile import add_dep_helper

T, D, G, NCORES = 8192, 1024, 8, 8
TPC = T // NCORES            # tokens per core
RT = TPC // 128              # row tiles per core (8)
KT = D // 128                # contraction tiles (8)
NSLOTS = 3
PATTERN = (0, 0, 0, 1, 1, 1, 2, 2)   # row-tile -> weight slot
LADDER_DEPTH = int(os.environ.get("K_LADDER", "4"))
WARMUP_MMS = int(os.environ.get("K_WARMUP", "6"))

CDT = mybir.dt.bfloat16      # compute dtype on device
NP_CDT = ml_dtypes.bfloat16
ODT = mybir.dt.bfloat16      # device output dtype (host upcasts)
NP_ODT = ml_dtypes.bfloat16

_PROG = None
LAST_RESULTS = None          # test harness reads exec_time_ns from here


def _ins(x):
    return getattr(x, "ins", x)


def _build_program():
    nc = bacc.Bacc("TRN2", target_bir_lowering=False, debug=False,
                   num_devices=NCORES)
    xt_d = nc.dram_tensor("xt", [D, TPC], CDT, kind="ExternalInput")
    w_d = nc.dram_tensor("w", [NSLOTS, D, D], CDT, kind="ExternalInput")
    o_d = nc.dram_tensor("o", [TPC, D], ODT, kind="ExternalOutput")

    with tile.TileContext(nc) as tc, ExitStack() as ctx:
        xt_pool = ctx.enter_context(tc.tile_pool(name="xt", bufs=1))
        w_pool = ctx.enter_context(tc.tile_pool(name="w", bufs=1))
        ps_pool = ctx.enter_context(
            tc.tile_pool(name="ps", bufs=int(os.environ.get("K_PSBUFS", "7")),
                         space=bass.MemorySpace.PSUM))
        out_pool = ctx.enter_context(tc.tile_pool(name="out", bufs=4))
        warm_pool = ctx.enter_context(tc.tile_pool(name="warm", bufs=1))

        # --- PE warmup: junk matmuls so HAM un-throttles while the first
        # real tiles are still in flight.
        if WARMUP_MMS:
            wt = warm_pool.tile([128, 512], CDT, tag="warm_sb")
            nc.gpsimd.memset(wt[:], 0)
            wps_pool = ctx.enter_context(
                tc.tile_pool(name="wps", bufs=1, space=bass.MemorySpace.PSUM))
            wps = wps_pool.tile([128, 512], mybir.dt.float32, tag="warm_ps")
            for _ in range(WARMUP_MMS):
                nc.tensor.matmul(wps[:], wt[:, 0:128], wt[:],
                                 start=True, stop=True)

        # --- Loads, in exact consume order, rate-limited by a ladder.
        load_insts = []

        def ladder_dma(dst_ap, src_ap):
            inst = nc.sync.dma_start(dst_ap, src_ap)
            if len(load_insts) >= LADDER_DEPTH:
                add_dep_helper(_ins(inst), _ins(load_insts[-LADDER_DEPTH]),
                               sync=True, reason="load ladder")
            load_insts.append(inst)
            return inst

        xt_sb = [None] * KT
        w_sb = {s: [None] * KT for s in range(NSLOTS)}

        def load_xt(k):
            t = xt_pool.tile([128, TPC], CDT, tag=f"xt{k}")
            ladder_dma(t[:], xt_d[k * 128:(k + 1) * 128, :])
            xt_sb[k] = t

        def load_w(s, k):
            t = w_pool.tile([128, D], CDT, tag=f"w{s}_{k}")
            ladder_dma(t[:], w_d[s, k * 128:(k + 1) * 128, :])
            w_sb[s][k] = t

        for k in range(KT):          # wave 0: xt and w0 interleaved
            load_xt(k)
            load_w(0, k)
        for s in range(1, NSLOTS):   # later slots follow in the ladder
            for k in range(KT):
                load_w(s, k)

        # --- Compute: per row tile, two 512-wide output halves, each an
        # 8-matmul K-chain into one PSUM bank.
        for rt in range(RT):
            s = PATTERN[rt]
            ot = out_pool.tile([128, D], ODT, tag="ot")
            for oh in range(2):
                ps = ps_pool.tile([128, 512], mybir.dt.float32, tag="ps")
                for k in range(KT):
                    nc.tensor.matmul(
                        ps[:],
                        xt_sb[k][:, rt * 128:(rt + 1) * 128],
                        w_sb[s][k][:, oh * 512:(oh + 1) * 512],
                        start=(k == 0),
                        stop=(k == KT - 1),
                    )
                nc.vector.tensor_copy(ot[:, oh * 512:(oh + 1) * 512], ps[:])
            # store on the scalar HWDGE ring so it can't stall load waits
            nc.scalar.dma_start(o_d[rt * 128:(rt + 1) * 128, :], ot[:])

    nc.compile()
    return nc


def _get_program():
    global _PROG
    if _PROG is None:
        _PROG = _build_program()
    return _PROG


def _solve_parts(tiles_per_expert):
    """Decompose per-expert tile counts into 16 parts of 3 tiles and 8
    parts of 2 tiles. Returns (threes, twos) as lists of expert ids, or
    None if infeasible."""
    t = list(tiles_per_expert)
    f = [c % 2 for c in t]              # number of 3-parts per expert
    if any(3 * f[g] > t[g] for g in range(len(t))):
        return None
    h = [(t[g] - 3 * f[g]) // 2 for g in range(len(t))]
    # each f+=2 converts three 2-parts into two 3-parts
    while sum(h) > 8:
        g = max(range(len(t)), key=lambda i: h[i])
        if h[g] < 3:
            return None
        f[g] += 2
        h[g] -= 3
    if sum(h) != 8 or sum(f) != 16:
        return None
    threes, twos = [], []
    for g in range(len(t)):
        threes += [g] * f[g]
        twos += [g] * h[g]
    return threes, twos


def _numpy_fallback(hidden_states, weight, counts):
    out = np.empty((hidden_states.shape[0], weight.shape[2]), np.float32)
    start = 0
    for g in range(weight.shape[0]):
        end = start + int(counts[g])
        out[start:end] = hidden_states[start:end].astype(np.float32) @ \
            weight[g].astype(np.float32)
        start = end
    return out


def kernel(hidden_states, weight, tokens_per_expert):
    counts = np.asarray(tokens_per_expert).astype(np.int64)
    out_dtype = hidden_states.dtype

    ok = (hidden_states.shape == (T, D) and weight.shape == (G, D, D)
          and counts.shape == (G,) and counts.sum() == T
          and np.all(counts % 128 == 0) and np.all(counts >= 0))
    parts = _solve_parts(counts // 128) if ok else None
    if parts is None:
        return _numpy_fallback(hidden_states, weight, counts).astype(out_dtype)
    threes, twos = parts

    # Global preprocessing: transpose+cast activations once, cast weights.
    ht = np.ascontiguousarray(
        hidden_states.astype(NP_CDT).T)          # [D, T] bf16
    wc = weight.astype(NP_CDT)                    # [G, D, D] bf16

    # Per-expert global row offsets; consume tiles in order.
    expert_row = dict(
        (g, int(o)) for g, o in enumerate(np.concatenate(
            [[0], np.cumsum(counts)[:-1]])))

    in_maps = []
    core_rows = []       # per core: list of (global_row_start, n_rows)
    for c in range(NCORES):
        part_list = [(threes[2 * c], 3 * 128), (threes[2 * c + 1], 3 * 128),
                     (twos[c], 2 * 128)]
        spans = []
        for g, nrows in part_list:
            r0 = expert_row[g]
            expert_row[g] = r0 + nrows
            spans.append((r0, nrows))
        core_rows.append(spans)
        xt_c = np.concatenate(
            [ht[:, r0:r0 + n] for r0, n in spans], axis=1)
        xt_c = np.ascontiguousarray(xt_c)
        w_c = np.ascontiguousarray(
            np.stack([wc[g] for g, _ in part_list]))
        in_maps.append({"xt": xt_c, "w": w_c})

    nc = _get_program()
    global LAST_RESULTS
    LAST_RESULTS = run_bass_kernel_spmd(nc, in_maps, list(range(NCORES)))

    out = np.empty((T, D), np.float32)
    for c in range(NCORES):
        o_c = np.asarray(LAST_RESULTS.results[c]["o"]).astype(np.float32)
        r = 0
        for r0, n in core_rows[c]:
            out[r0:r0 + n] = o_c[r:r + n]
            r += n
    return out.astype(out_dtype, copy=False)


# revision 7
# speedup vs baseline: 1.2710x; 1.1312x over previous
"""Grouped linear (MoE routed GEMM) on 8 Trainium2 NeuronCores.

out[t] = hidden_states[t] @ weight[g(t)] where g(t) is the expert owning
token t (contiguous groups sized by tokens_per_expert).

Strategy (expert-parallel, token-balanced):
  - All group sizes are multiples of 128, so work = 64 row-tiles of 128
    tokens. Each core gets exactly 8 row-tiles (1024 tokens).
  - SPMD requires one program for all cores, so every core runs the same
    static slot pattern [0,0,0,1,1,1,2,2]: 3 weight slots covering 3/3/2
    row-tiles. The host decomposes the per-expert tile counts into
    sixteen 3-tile parts + eight 2-tile parts and assigns (expert ->
    core,slot); each core receives its 1024 tokens pre-transposed plus a
    [3,1024,1024] stacked weight tensor.
  - Device: bf16 matmuls (fp32 PSUM accumulate), K-contiguous chains of
    8 matmuls per [128,512] output tile. Loads are emitted in exact
    consume order and rate-limited by a dependency ladder so tiles
    arrive progressively (SDMA round-robins across queues at packet
    granularity; without the ladder every transfer lands at the same
    late time and the PE idles for ~12us). Stores go on the scalar
    engine's HWDGE ring so they never stall behind load waits.
"""

import os
import numpy as np
import ml_dtypes
from contextlib import ExitStack

import concourse.bass as bass
import concourse.tile as tile
from concourse import bacc, mybir
from concourse.bass_utils import run_bass_kernel_spmd
from concourse.tile import add_dep_helper

T, D, G, NCORES = 8192, 1024, 8, 8
TPC = T // NCORES            # tokens per core
RT = TPC // 128              # row tiles per core (8)
KT = D // 128                # contraction tiles (8)
NSLOTS = 3
PATTERN = (0, 0, 0, 1, 1, 1, 2, 2)   # row-tile -> weight slot
LADDER_DEPTH = int(os.environ.get("K_LADDER", "4"))
WARMUP_MMS = int(os.environ.get("K_WARMUP", "6"))

CDT = mybir.dt.bfloat16      # compute dtype on device
NP_CDT = ml_dtypes.bfloat16
ODT = mybir.dt.bfloat16      # device output dtype (host upcasts)
NP_ODT = ml_dtypes.bfloat16

_PROG = None
LAST_RESULTS = None          # test harness reads exec_time_ns from here


def _ins(x):
    return getattr(x, "ins", x)


def _build_program():
    """Device program (identical on all 8 cores).

    DRAM inputs are host-packed, partition-major, in consume order:
      wv0 [4, 128, 4096]: batch b = xt_{2b} | w0_{2b} | xt_{2b+1} | w0_{2b+1}
                          (column blocks of 1024; partition p = K-row p)
      wv1 [128, 8192]:    slot-1 weight, k-tile k at cols k*1024
      wv2 [128, 8192]:    slot-2 weight, likewise
    Each wave is one large contiguous DMA (8-16KB per-partition lines) so
    data arrives in descriptor order; wv1/wv2 issue is gated on compute
    progress so they never steal bandwidth from the startup ramp.
    """
    nc = bacc.Bacc("TRN2", target_bir_lowering=False, debug=False,
                   num_devices=NCORES)
    wv0_d = nc.dram_tensor("wv0", [4, 128, 4 * 1024], CDT,
                           kind="ExternalInput")
    wv1_d = nc.dram_tensor("wv1", [128, KT * 1024], CDT,
                           kind="ExternalInput")
    wv2_d = nc.dram_tensor("wv2", [128, KT * 1024], CDT,
                           kind="ExternalInput")
    o_d = nc.dram_tensor("o", [TPC, D], ODT, kind="ExternalOutput")

    with tile.TileContext(nc) as tc, ExitStack() as ctx:
        ld_pool = ctx.enter_context(tc.tile_pool(name="ld", bufs=1))
        ps_pool = ctx.enter_context(
            tc.tile_pool(name="ps", bufs=int(os.environ.get("K_PSBUFS", "7")),
                         space=bass.MemorySpace.PSUM))
        out_pool = ctx.enter_context(tc.tile_pool(name="out", bufs=4))
        warm_pool = ctx.enter_context(tc.tile_pool(name="warm", bufs=1))

        # --- PE warmup: junk matmuls so HAM un-throttles while the first
        # real tiles are still in flight.
        if WARMUP_MMS:
            wt = warm_pool.tile([128, 512], CDT, tag="warm_sb")
            nc.gpsimd.memset(wt[:], 0)
            wps_pool = ctx.enter_context(
                tc.tile_pool(name="wps", bufs=1, space=bass.MemorySpace.PSUM))
            wps = wps_pool.tile([128, 512], mybir.dt.float32, tag="warm_ps")
            for _ in range(WARMUP_MMS):
                nc.tensor.matmul(wps[:], wt[:, 0:128], wt[:],
                                 start=True, stop=True)

        # --- Wave-0 batches, laddered so arrival is progressive.
        b_sb = []
        b_dma = []
        for b in range(4):
            t = ld_pool.tile([128, 4 * 1024], CDT, tag=f"b{b}")
            inst = nc.sync.dma_start(t[:], wv0_d[b])
            if b >= 2:
                add_dep_helper(_ins(inst), _ins(b_dma[b - 2]),
                               sync=True, reason="wave0 ladder")
            b_sb.append(t)
            b_dma.append(inst)
        wv1_sb = ld_pool.tile([128, KT * 1024], CDT, tag="wv1")
        wv1_dma = nc.sync.dma_start(wv1_sb[:], wv1_d[:])
        wv2_sb = ld_pool.tile([128, KT * 1024], CDT, tag="wv2")
        wv2_dma = nc.sync.dma_start(wv2_sb[:], wv2_d[:])

        # Accessors: lhsT [128,128] and rhs [128,512] slices per (k, ...).
        def xt_ap(k, rt):
            t = b_sb[k // 2]
            base = (k % 2) * 2048
            return t[:, base + rt * 128: base + (rt + 1) * 128]

        def w_ap(s, k, oh):
            if s == 0:
                t = b_sb[k // 2]
                base = (k % 2) * 2048 + 1024
                return t[:, base + oh * 512: base + (oh + 1) * 512]
            t = wv1_sb if s == 1 else wv2_sb
            return t[:, k * 1024 + oh * 512: k * 1024 + (oh + 1) * 512]

        # --- Compute: per row tile, two 512-wide output halves, each an
        # 8-matmul K-chain into one PSUM bank.
        trigger_mm = {}
        for rt in range(RT):
            s = PATTERN[rt]
            ot = out_pool.tile([128, D], ODT, tag="ot")
            for oh in range(2):
                ps = ps_pool.tile([128, 512], mybir.dt.float32, tag="ps")
                for k in range(KT):
                    mm = nc.tensor.matmul(
                        ps[:],
                        xt_ap(k, rt),
                        w_ap(s, k, oh),
                        start=(k == 0),
                        stop=(k == KT - 1),
                    )
                    if (rt, oh, k) == (0, 0, 2):
                        trigger_mm["wv1"] = mm
                    if (rt, oh, k) == (3, 0, 0):
                        trigger_mm["wv2"] = mm
                nc.vector.tensor_copy(ot[:, oh * 512:(oh + 1) * 512], ps[:])
            # store on the scalar HWDGE ring so it can't stall load waits
            nc.scalar.dma_start(o_d[rt * 128:(rt + 1) * 128, :], ot[:])

        # Gate late weight waves on compute progress (not on DMA chains):
        # they start streaming while wave-0's tail is in flight but can't
        # front-run the whole ramp.
        add_dep_helper(_ins(wv1_dma), _ins(trigger_mm["wv1"]),
                       sync=True, reason="wv1 after slot0 ramp")
        add_dep_helper(_ins(wv2_dma), _ins(trigger_mm["wv2"]),
                       sync=True, reason="wv2 after slot1 start")

    nc.compile()
    return nc


def _get_program():
    global _PROG
    if _PROG is None:
        _PROG = _build_program()
    return _PROG


def _solve_parts(tiles_per_expert):
    """Decompose per-expert tile counts into 16 parts of 3 tiles and 8
    parts of 2 tiles. Returns (threes, twos) as lists of expert ids, or
    None if infeasible."""
    t = list(tiles_per_expert)
    f = [c % 2 for c in t]              # number of 3-parts per expert
    if any(3 * f[g] > t[g] for g in range(len(t))):
        return None
    h = [(t[g] - 3 * f[g]) // 2 for g in range(len(t))]
    # each f+=2 converts three 2-parts into two 3-parts
    while sum(h) > 8:
        g = max(range(len(t)), key=lambda i: h[i])
        if h[g] < 3:
            return None
        f[g] += 2
        h[g] -= 3
    if sum(h) != 8 or sum(f) != 16:
        return None
    threes, twos = [], []
    for g in range(len(t)):
        threes += [g] * f[g]
        twos += [g] * h[g]
    return threes, twos


def _numpy_fallback(hidden_states, weight, counts):
    out = np.empty((hidden_states.shape[0], weight.shape[2]), np.float32)
    start = 0
    for g in range(weight.shape[0]):
        end = start + int(counts[g])
        out[start:end] = hidden_states[start:end].astype(np.float32) @ \
            weight[g].astype(np.float32)
        start = end
    return out


def kernel(hidden_states, weight, tokens_per_expert):
    counts = np.asarray(tokens_per_expert).astype(np.int64)
    out_dtype = hidden_states.dtype

    ok = (hidden_states.shape == (T, D) and weight.shape == (G, D, D)
          and counts.shape == (G,) and counts.sum() == T
          and np.all(counts % 128 == 0) and np.all(counts >= 0))
    parts = _solve_parts(counts // 128) if ok else None
    if parts is None:
        return _numpy_fallback(hidden_states, weight, counts).astype(out_dtype)
    threes, twos = parts

    # Global preprocessing: transpose+cast activations once, cast weights.
    ht = np.ascontiguousarray(
        hidden_states.astype(NP_CDT).T)          # [D, T] bf16
    wc = weight.astype(NP_CDT)                    # [G, D, D] bf16

    # Per-expert global row offsets; consume tiles in order.
    expert_row = dict(
        (g, int(o)) for g, o in enumerate(np.concatenate(
            [[0], np.cumsum(counts)[:-1]])))

    in_maps = []
    core_rows = []       # per core: list of (global_row_start, n_rows)
    for c in range(NCORES):
        part_list = [(threes[2 * c], 3 * 128), (threes[2 * c + 1], 3 * 128),
                     (twos[c], 2 * 128)]
        spans = []
        for g, nrows in part_list:
            r0 = expert_row[g]
            expert_row[g] = r0 + nrows
            spans.append((r0, nrows))
        core_rows.append(spans)
        # xt_c: [D, TPC] activations (pre-transposed); k-tile k = rows
        # k*128..k*128+127.
        xt_c = np.concatenate(
            [ht[:, r0:r0 + n] for r0, n in spans], axis=1)
        w_slots = [wc[g] for g, _ in part_list]   # 3 x [D, D]

        # wv0 [4, 128, 4096]: batch b packs k-tiles 2b,2b+1 of xt and w0,
        # partition-major: wv0[b, p] = xt[2b*128+p,:] | w0[2b*128+p,:]
        #                              | xt[(2b+1)*128+p,:] | w0[...]
        xt_k = xt_c.reshape(KT, 128, TPC)
        w0_k = w_slots[0].reshape(KT, 128, D)
        wv0 = np.empty((4, 128, 4 * 1024), dtype=NP_CDT)
        for b in range(4):
            wv0[b, :, 0:1024] = xt_k[2 * b]
            wv0[b, :, 1024:2048] = w0_k[2 * b]
            wv0[b, :, 2048:3072] = xt_k[2 * b + 1]
            wv0[b, :, 3072:4096] = w0_k[2 * b + 1]
        # wv1/wv2 [128, 8192]: row p = concat_k W[k*128+p, :]
        wv1 = np.ascontiguousarray(
            w_slots[1].reshape(KT, 128, D).transpose(1, 0, 2).reshape(
                128, KT * D))
        wv2 = np.ascontiguousarray(
            w_slots[2].reshape(KT, 128, D).transpose(1, 0, 2).reshape(
                128, KT * D))
        in_maps.append({"wv0": wv0, "wv1": wv1, "wv2": wv2})

    nc = _get_program()
    global LAST_RESULTS
    LAST_RESULTS = run_bass_kernel_spmd(nc, in_maps, list(range(NCORES)))

    out = np.empty((T, D), np.float32)
    for c in range(NCORES):
        o_c = np.asarray(LAST_RESULTS.results[c]["o"]).astype(np.float32)
        r = 0
        for r0, n in core_rows[c]:
            out[r0:r0 + n] = o_c[r:r + n]
            r += n
    return out.astype(out_dtype, copy=False)


# revision 10
# speedup vs baseline: 1.3196x; 1.0382x over previous
"""Grouped linear (MoE routed GEMM) on 8 Trainium2 NeuronCores.

out[t] = hidden_states[t] @ weight[g(t)] where g(t) is the expert owning
token t (contiguous groups sized by tokens_per_expert).

Strategy (expert-parallel, token-balanced):
  - All group sizes are multiples of 128, so work = 64 row-tiles of 128
    tokens. Each core gets exactly 8 row-tiles (1024 tokens).
  - SPMD requires one program for all cores, so every core runs the same
    static slot pattern [0,0,0,1,1,1,2,2]: 3 weight slots covering 3/3/2
    row-tiles. The host decomposes the per-expert tile counts into
    sixteen 3-tile parts + eight 2-tile parts and assigns (expert ->
    core,slot); each core receives its 1024 tokens pre-transposed plus a
    [3,1024,1024] stacked weight tensor.
  - Device: bf16 matmuls (fp32 PSUM accumulate), K-contiguous chains of
    8 matmuls per [128,512] output tile. Loads are emitted in exact
    consume order and rate-limited by a dependency ladder so tiles
    arrive progressively (SDMA round-robins across queues at packet
    granularity; without the ladder every transfer lands at the same
    late time and the PE idles for ~12us). Stores go on the scalar
    engine's HWDGE ring so they never stall behind load waits.
"""

import os
import numpy as np
import ml_dtypes
from contextlib import ExitStack

import concourse.bass as bass
import concourse.tile as tile
from concourse import bacc, mybir
from concourse.bass_utils import run_bass_kernel_spmd
from concourse.tile import add_dep_helper

T, D, G, NCORES = 8192, 1024, 8, 8
TPC = T // NCORES            # tokens per core
RT = TPC // 128              # row tiles per core (8)
KT = D // 128                # contraction tiles (8)
NSLOTS = 3
PATTERN = (0, 0, 0, 1, 1, 1, 2, 2)   # row-tile -> weight slot
LADDER_DEPTH = int(os.environ.get("K_LADDER", "4"))
WARMUP_MMS = int(os.environ.get("K_WARMUP", "6"))

CDT = mybir.dt.bfloat16      # compute dtype on device
NP_CDT = ml_dtypes.bfloat16
ODT = mybir.dt.bfloat16      # device output dtype (host upcasts)
NP_ODT = ml_dtypes.bfloat16

_PROG = None
LAST_RESULTS = None          # test harness reads exec_time_ns from here


def _ins(x):
    return getattr(x, "ins", x)


def _build_program():
    """Device program (identical on all 8 cores).

    DRAM inputs are host-packed, partition-major, in consume order:
      wv0 [4, 128, 4096]: batch b = xt_{2b} | w0_{2b} | xt_{2b+1} | w0_{2b+1}
                          (column blocks of 1024; partition p = K-row p)
      wv1 [128, 8192]:    slot-1 weight, k-tile k at cols k*1024
      wv2 [128, 8192]:    slot-2 weight, likewise
    Each wave is one large contiguous DMA (8-16KB per-partition lines) so
    data arrives in descriptor order; wv1/wv2 issue is gated on compute
    progress so they never steal bandwidth from the startup ramp.
    """
    nc = bacc.Bacc("TRN2", target_bir_lowering=False, debug=False,
                   num_devices=NCORES)
    wv0_d = nc.dram_tensor("wv0", [4, 128, 4 * 1024], CDT,
                           kind="ExternalInput")
    wv1_d = nc.dram_tensor("wv1", [128, KT * 1024], CDT,
                           kind="ExternalInput")
    wv2_d = nc.dram_tensor("wv2", [128, KT * 1024], CDT,
                           kind="ExternalInput")
    o_d = nc.dram_tensor("o", [TPC, D], ODT, kind="ExternalOutput")

    with tile.TileContext(nc) as tc, ExitStack() as ctx:
        ld_pool = ctx.enter_context(tc.tile_pool(name="ld", bufs=1))
        ps_pool = ctx.enter_context(
            tc.tile_pool(name="ps", bufs=int(os.environ.get("K_PSBUFS", "7")),
                         space=bass.MemorySpace.PSUM))
        out_pool = ctx.enter_context(tc.tile_pool(name="out", bufs=4))
        warm_pool = ctx.enter_context(tc.tile_pool(name="warm", bufs=1))

        # --- PE warmup: junk matmuls so HAM un-throttles while the first
        # real tiles are still in flight.
        if WARMUP_MMS:
            wt = warm_pool.tile([128, 512], CDT, tag="warm_sb")
            nc.gpsimd.memset(wt[:], 0)
            wps_pool = ctx.enter_context(
                tc.tile_pool(name="wps", bufs=1, space=bass.MemorySpace.PSUM))
            wps = wps_pool.tile([128, 512], mybir.dt.float32, tag="warm_ps")
            for _ in range(WARMUP_MMS):
                nc.tensor.matmul(wps[:], wt[:, 0:128], wt[:],
                                 start=True, stop=True)

        # --- Wave-0 batches, laddered so arrival is progressive.
        b_sb = []
        b_dma = []
        for b in range(4):
            t = ld_pool.tile([128, 4 * 1024], CDT, tag=f"b{b}")
            inst = nc.sync.dma_start(t[:], wv0_d[b])
            if b >= 2:
                add_dep_helper(_ins(inst), _ins(b_dma[b - 2]),
                               sync=True, reason="wave0 ladder")
            b_sb.append(t)
            b_dma.append(inst)
        # Late weight waves, split in two halves (k 0-3 / k 4-7) so the
        # first half's completion unblocks compute without waiting for the
        # whole 2MB + receipt latency.
        HALF = KT // 2 * 1024
        wv1_sb = ld_pool.tile([128, KT * 1024], CDT, tag="wv1")
        wv1_dma = [nc.sync.dma_start(wv1_sb[:, h * HALF:(h + 1) * HALF],
                                     wv1_d[:, h * HALF:(h + 1) * HALF])
                   for h in range(2)]
        wv2_sb = ld_pool.tile([128, KT * 1024], CDT, tag="wv2")
        wv2_dma = [nc.sync.dma_start(wv2_sb[:, h * HALF:(h + 1) * HALF],
                                     wv2_d[:, h * HALF:(h + 1) * HALF])
                   for h in range(2)]

        # Accessors: lhsT [128,128] and rhs [128,512] slices per (k, ...).
        def xt_ap(k, rt):
            t = b_sb[k // 2]
            base = (k % 2) * 2048
            return t[:, base + rt * 128: base + (rt + 1) * 128]

        def w_ap(s, k, oh):
            if s == 0:
                t = b_sb[k // 2]
                base = (k % 2) * 2048 + 1024
                return t[:, base + oh * 512: base + (oh + 1) * 512]
            t = wv1_sb if s == 1 else wv2_sb
            return t[:, k * 1024 + oh * 512: k * 1024 + (oh + 1) * 512]

        # --- Compute: per row tile, two 512-wide output halves, each an
        # 8-matmul K-chain into one PSUM bank.
        trigger_mm = {}
        for rt in range(RT):
            s = PATTERN[rt]
            ot = out_pool.tile([128, D], ODT, tag="ot")
            for oh in range(2):
                ps = ps_pool.tile([128, 512], mybir.dt.float32, tag="ps")
                for k in range(KT):
                    mm = nc.tensor.matmul(
                        ps[:],
                        xt_ap(k, rt),
                        w_ap(s, k, oh),
                        start=(k == 0),
                        stop=(k == KT - 1),
                    )
                    if (rt, oh, k) == (0, 0, 0):
                        trigger_mm["wv1a"] = mm
                    if (rt, oh, k) == (0, 0, 4):
                        trigger_mm["wv1b"] = mm
                    if (rt, oh, k) == (3, 0, 0):
                        trigger_mm["wv2a"] = mm
                    if (rt, oh, k) == (3, 0, 4):
                        trigger_mm["wv2b"] = mm
                nc.vector.tensor_copy(ot[:, oh * 512:(oh + 1) * 512], ps[:])
                # store each half as soon as it's copied (scalar HWDGE
                # ring, so stores never stall behind load waits)
                nc.scalar.dma_start(
                    o_d[rt * 128:(rt + 1) * 128, oh * 512:(oh + 1) * 512],
                    ot[:, oh * 512:(oh + 1) * 512])

        # Gate late weight waves on compute progress (not on DMA chains):
        # they start streaming while wave-0's tail is in flight but can't
        # front-run the whole ramp.
        add_dep_helper(_ins(wv1_dma[0]), _ins(trigger_mm["wv1a"]),
                       sync=True, reason="wv1a after slot0 start")
        add_dep_helper(_ins(wv1_dma[1]), _ins(trigger_mm["wv1b"]),
                       sync=True, reason="wv1b after slot0 mid")
        add_dep_helper(_ins(wv2_dma[0]), _ins(trigger_mm["wv2a"]),
                       sync=True, reason="wv2a after slot1 start")
        add_dep_helper(_ins(wv2_dma[1]), _ins(trigger_mm["wv2b"]),
                       sync=True, reason="wv2b after slot1 mid")

    nc.compile()
    return nc


def _get_program():
    global _PROG
    if _PROG is None:
        _PROG = _build_program()
    return _PROG


def _solve_parts(tiles_per_expert):
    """Decompose per-expert tile counts into 16 parts of 3 tiles and 8
    parts of 2 tiles. Returns (threes, twos) as lists of expert ids, or
    None if infeasible."""
    t = list(tiles_per_expert)
    f = [c % 2 for c in t]              # number of 3-parts per expert
    if any(3 * f[g] > t[g] for g in range(len(t))):
        return None
    h = [(t[g] - 3 * f[g]) // 2 for g in range(len(t))]
    # each f+=2 converts three 2-parts into two 3-parts
    while sum(h) > 8:
        g = max(range(len(t)), key=lambda i: h[i])
        if h[g] < 3:
            return None
        f[g] += 2
        h[g] -= 3
    if sum(h) != 8 or sum(f) != 16:
        return None
    threes, twos = [], []
    for g in range(len(t)):
        threes += [g] * f[g]
        twos += [g] * h[g]
    return threes, twos


def _numpy_fallback(hidden_states, weight, counts):
    out = np.empty((hidden_states.shape[0], weight.shape[2]), np.float32)
    start = 0
    for g in range(weight.shape[0]):
        end = start + int(counts[g])
        out[start:end] = hidden_states[start:end].astype(np.float32) @ \
            weight[g].astype(np.float32)
        start = end
    return out


def kernel(hidden_states, weight, tokens_per_expert):
    counts = np.asarray(tokens_per_expert).astype(np.int64)
    out_dtype = hidden_states.dtype

    ok = (hidden_states.shape == (T, D) and weight.shape == (G, D, D)
          and counts.shape == (G,) and counts.sum() == T
          and np.all(counts % 128 == 0) and np.all(counts >= 0))
    parts = _solve_parts(counts // 128) if ok else None
    if parts is None:
        return _numpy_fallback(hidden_states, weight, counts).astype(out_dtype)
    threes, twos = parts

    # Global preprocessing: transpose+cast activations once, cast weights.
    ht = np.ascontiguousarray(
        hidden_states.astype(NP_CDT).T)          # [D, T] bf16
    wc = weight.astype(NP_CDT)                    # [G, D, D] bf16

    # Per-expert global row offsets; consume tiles in order.
    expert_row = dict(
        (g, int(o)) for g, o in enumerate(np.concatenate(
            [[0], np.cumsum(counts)[:-1]])))

    in_maps = []
    core_rows = []       # per core: list of (global_row_start, n_rows)
    for c in range(NCORES):
        part_list = [(threes[2 * c], 3 * 128), (threes[2 * c + 1], 3 * 128),
                     (twos[c], 2 * 128)]
        spans = []
        for g, nrows in part_list:
            r0 = expert_row[g]
            expert_row[g] = r0 + nrows
            spans.append((r0, nrows))
        core_rows.append(spans)
        # xt_c: [D, TPC] activations (pre-transposed); k-tile k = rows
        # k*128..k*128+127.
        xt_c = np.concatenate(
            [ht[:, r0:r0 + n] for r0, n in spans], axis=1)
        w_slots = [wc[g] for g, _ in part_list]   # 3 x [D, D]

        # wv0 [4, 128, 4096]: batch b packs k-tiles 2b,2b+1 of xt and w0,
        # partition-major: wv0[b, p] = xt[2b*128+p,:] | w0[2b*128+p,:]
        #                              | xt[(2b+1)*128+p,:] | w0[...]
        xt_k = xt_c.reshape(KT, 128, TPC)
        w0_k = w_slots[0].reshape(KT, 128, D)
        wv0 = np.empty((4, 128, 4 * 1024), dtype=NP_CDT)
        for b in range(4):
            wv0[b, :, 0:1024] = xt_k[2 * b]
            wv0[b, :, 1024:2048] = w0_k[2 * b]
            wv0[b, :, 2048:3072] = xt_k[2 * b + 1]
            wv0[b, :, 3072:4096] = w0_k[2 * b + 1]
        # wv1/wv2 [128, 8192]: row p = concat_k W[k*128+p, :]
        wv1 = np.ascontiguousarray(
            w_slots[1].reshape(KT, 128, D).transpose(1, 0, 2).reshape(
                128, KT * D))
        wv2 = np.ascontiguousarray(
            w_slots[2].reshape(KT, 128, D).transpose(1, 0, 2).reshape(
                128, KT * D))
        in_maps.append({"wv0": wv0, "wv1": wv1, "wv2": wv2})

    nc = _get_program()
    global LAST_RESULTS
    LAST_RESULTS = run_bass_kernel_spmd(nc, in_maps, list(range(NCORES)))

    out = np.empty((T, D), np.float32)
    for c in range(NCORES):
        o_c = np.asarray(LAST_RESULTS.results[c]["o"]).astype(np.float32)
        r = 0
        for r0, n in core_rows[c]:
            out[r0:r0 + n] = o_c[r:r + n]
            r += n
    return out.astype(out_dtype, copy=False)
